# revision 1
# baseline (speedup 1.0000x reference)
"""Trainium2 Bass kernel for nn_LogicalOperatorFusion.

Semantics (matches the jax reference):
  fv = field_vectors                                  [B=1024, NF=64, H=512] f32
  fv[:, not_idx] = tanh(fv[:, not_idx] @ not_W + not_b)
  cat_and = concat(fv[:, and_pairs[:,0]], fv[:, and_pairs[:,1]], -1)   [B,16,1024]
  fused_and = relu(cat_and @ and_W1 + and_b1) @ and_W2 + and_b2        [B,16,512]
  cat_or  = ... same with or_pairs (8 pairs) ...                       [B,8,512]
  out = concat([fused_and, fused_or, fv[:, leftover_idx]], axis=1)     [B,40,512]

Strategy:
  - Data parallel: batch sharded 8 ways (128 rows/core), weights replicated.
  - Host pre-transposes activations so the PE stationary operand (lhsT =
    X^T chunks, [K=128 h, M=128 rows]) arrives DMA-ready; weights are the
    N=512 moving operand.  Only the hidden layer h needs on-chip transposes
    (4 PE identity-transposes per pair).
  - Biases are applied as rank-1 (K=1) ones (x) b matmuls accumulated in PSUM,
    so ACT does single-pass relu/tanh PSUM->SBUF.
  - Untouched leftover fields (not in not_idx) never touch the device; the
    host copies them straight from the input.
"""

import sys

for p in ("/opt/trn_rl_repo",):
    if p not in sys.path:
        sys.path.insert(0, p)

import numpy as np
import ml_dtypes

BF16_NP = ml_dtypes.bfloat16

import concourse.bacc as bacc
import concourse.mybir as mybir
from concourse.bass_utils import run_bass_kernel_spmd
from concourse.tile import TileContext

B, NF, H = 1024, 64, 512
NCORES = 8
BS = B // NCORES  # 128 rows per core
NAND, NOR, NNOT = 16, 8, 8
NPAIR = NAND + NOR  # 24
NOUT = NPAIR + NNOT  # 32 device output slots
KC1 = (2 * H) // 128  # 8 k-chunks for layer 1
KC2 = H // 128  # 4 k-chunks for layer 2 / not
F32 = mybir.dt.float32
BF16 = mybir.dt.bfloat16

TRACE = False  # test.py sets this for profiled runs
LAST_RESULT = None  # BassKernelResults of the last run (for test.py)

_NC = None  # cached traced Bass program


def _build_nc():
    """Trace the per-core Bass program (identical on all 8 cores)."""
    nc = bacc.Bacc("TRN2", target_bir_lowering=False)

    pairs_in = nc.dram_tensor("pairs_in", [NPAIR, 128, 2 * H], BF16, kind="ExternalInput")
    nots_in = nc.dram_tensor("nots_in", [NNOT, 128, H], BF16, kind="ExternalInput")
    and_w1t = nc.dram_tensor("and_w1t", [128, KC1 * H], BF16, kind="ExternalInput")
    or_w1t = nc.dram_tensor("or_w1t", [128, KC1 * H], BF16, kind="ExternalInput")
    and_w2t = nc.dram_tensor("and_w2t", [128, KC2 * H], BF16, kind="ExternalInput")
    or_w2t = nc.dram_tensor("or_w2t", [128, KC2 * H], BF16, kind="ExternalInput")
    not_wt = nc.dram_tensor("not_wt", [128, KC2 * H], BF16, kind="ExternalInput")
    biases_in = nc.dram_tensor("biases", [5, 1, H], BF16, kind="ExternalInput")
    b2bc_in = nc.dram_tensor("b2bc", [2, 128, H], F32, kind="ExternalInput")
    b1bc_in = nc.dram_tensor("b1bc", [2, 128, H], F32, kind="ExternalInput")
    ident_in = nc.dram_tensor("ident", [128, 128], BF16, kind="ExternalInput")
    out_d = nc.dram_tensor("out", [NOUT, 128, H], F32, kind="ExternalOutput")

    with TileContext(nc) as tc:
        with (
            tc.tile_pool(name="consts", bufs=1) as consts,
            tc.tile_pool(name="weights", bufs=1) as wpool,
            tc.tile_pool(name="pairin", bufs=4) as inpool,
            tc.tile_pool(name="notin", bufs=3) as notpool,
            tc.tile_pool(name="hid", bufs=3) as hpool,
            tc.tile_pool(name="hidT", bufs=3) as htpool,
            tc.tile_pool(name="outs", bufs=4) as opool,
            tc.tile_pool(name="psum", bufs=2, space="PSUM") as psum,
        ):
            ident = consts.tile([128, 128], BF16, tag="ident")
            nc.sync.dma_start(out=ident, in_=ident_in[:, :])
            ones = consts.tile([1, 128], BF16, tag="ones")
            nc.vector.memset(ones, 1.0)
            bt = []
            for i in range(5):  # and_b1, or_b1, and_b2, or_b2, not_b
                b = consts.tile([1, H], BF16, tag=f"bias{i}")
                nc.sync.dma_start(out=b, in_=biases_in[i, :, :])
                bt.append(b)
            b_not = bt[4]
            b2bc_and = consts.tile([128, H], F32, tag="b2bca")
            nc.sync.dma_start(out=b2bc_and, in_=b2bc_in[0, :, :])
            b2bc_or = consts.tile([128, H], F32, tag="b2bco")
            nc.sync.dma_start(out=b2bc_or, in_=b2bc_in[1, :, :])
            b2bc = {True: b2bc_and, False: b2bc_or}
            b1bc_and = consts.tile([128, H], F32, tag="b1bca")
            nc.sync.dma_start(out=b1bc_and, in_=b1bc_in[0, :, :])
            b1bc_or = consts.tile([128, H], F32, tag="b1bco")
            nc.sync.dma_start(out=b1bc_or, in_=b1bc_in[1, :, :])
            b1bc = {True: b1bc_and, False: b1bc_or}

            w1_and = wpool.tile([128, KC1 * H], BF16, tag="w1a")
            nc.sync.dma_start(out=w1_and, in_=and_w1t[:, :])
            w2_and = wpool.tile([128, KC2 * H], BF16, tag="w2a")
            nc.sync.dma_start(out=w2_and, in_=and_w2t[:, :])
            w1_or = wpool.tile([128, KC1 * H], BF16, tag="w1o")
            w2_or = wpool.tile([128, KC2 * H], BF16, tag="w2o")
            w_not = wpool.tile([128, KC2 * H], BF16, tag="wn")
            w1 = {True: w1_and, False: w1_or}
            w2 = {True: w2_and, False: w2_or}

            hstate = {}

            def emit_l1(j):
                is_and = j < NAND
                t = inpool.tile([128, 2 * H], BF16, tag="pt_in")
                nc.sync.dma_start(out=t, in_=pairs_in[j, :, :])
                ph = psum.tile([128, H], F32, tag="ps_h")
                for k in range(KC1):
                    nc.tensor.matmul(
                        ph,
                        t[:, k * 128 : (k + 1) * 128],
                        w1[is_and][:, k * H : (k + 1) * H],
                        start=(k == 0),
                        stop=(k == KC1 - 1),
                    )
                htmp = hpool.tile([128, H], F32, tag="htmp_sb")
                nc.vector.scalar_tensor_tensor(
                    out=htmp,
                    in0=ph,
                    scalar=0.0,
                    in1=b1bc[is_and],
                    op0=mybir.AluOpType.bypass,
                    op1=mybir.AluOpType.add,
                )
                h = hpool.tile([128, H], BF16, tag="h_sb")
                nc.scalar.activation(h, htmp, mybir.ActivationFunctionType.Relu)
                hstate[j] = h

            htstate = {}

            def emit_trans(j):
                h = hstate.pop(j)
                pt = psum.tile([128, H], BF16, tag="ps_t")
                for c in range(KC2):
                    nc.tensor.transpose(
                        pt[:, c * 128 : (c + 1) * 128],
                        h[:, c * 128 : (c + 1) * 128],
                        ident,
                    )
                ht = htpool.tile([128, H], BF16, tag="ht_sb")
                nc.vector.tensor_copy(out=ht, in_=pt)
                htstate[j] = ht

            def emit_l2(j):
                is_and = j < NAND
                ht = htstate.pop(j)
                po = psum.tile([128, H], F32, tag="ps_o")
                for c in range(KC2):
                    nc.tensor.matmul(
                        po,
                        ht[:, c * 128 : (c + 1) * 128],
                        w2[is_and][:, c * H : (c + 1) * H],
                        start=(c == 0),
                        stop=(c == KC2 - 1),
                    )
                ot = opool.tile([128, H], F32, tag="o_sb")
                nc.vector.scalar_tensor_tensor(
                    out=ot,
                    in0=po,
                    scalar=0.0,
                    in1=b2bc[is_and],
                    op0=mybir.AluOpType.bypass,
                    op1=mybir.AluOpType.add,
                )
                nc.sync.dma_start(out=out_d[j, :, :], in_=ot)

            def emit_not(j):
                t = notpool.tile([128, H], BF16, tag="nt_in")
                nc.sync.dma_start(out=t, in_=nots_in[j, :, :])
                pn = psum.tile([128, H], F32, tag="ps_n")
                nc.tensor.matmul(pn, ones, b_not, start=True, stop=False)
                for c in range(KC2):
                    nc.tensor.matmul(
                        pn,
                        t[:, c * 128 : (c + 1) * 128],
                        w_not[:, c * H : (c + 1) * H],
                        start=False,
                        stop=(c == KC2 - 1),
                    )
                ot = opool.tile([128, H], F32, tag="o_sb")
                nc.scalar.activation(ot, pn, mybir.ActivationFunctionType.Tanh)
                nc.sync.dma_start(out=out_d[NPAIR + j, :, :], in_=ot)

            # 2-stage software skew: between the transposes of job j-1 and
            # their dependent L2 matmuls (which wait on the DVE copy of hT),
            # the PE runs L1(j) and L2(j-2) — no PE stall on ACT or DVE.
            emit_l1(0)
            emit_l1(1)
            emit_trans(0)
            for j in range(2, NPAIR):
                if j == 8:  # or-weights needed from job 16; queue their DMAs now
                    nc.sync.dma_start(out=w1_or, in_=or_w1t[:, :])
                if j == 12:
                    nc.sync.dma_start(out=w2_or, in_=or_w2t[:, :])
                    nc.sync.dma_start(out=w_not, in_=not_wt[:, :])
                emit_l1(j)
                emit_trans(j - 1)
                emit_l2(j - 2)
            emit_trans(NPAIR - 1)
            emit_l2(NPAIR - 2)
            emit_l2(NPAIR - 1)
            for j in range(NNOT):
                emit_not(j)

    nc.compile()
    return nc


def _get_nc():
    global _NC
    if _NC is None:
        _NC = _build_nc()
    return _NC


def _w1_tiles(W):  # [2H, H] -> [128, KC1*H], tile[p, k*H+n] = W[k*128+p, n]
    return np.ascontiguousarray(
        W.reshape(KC1, 128, H).transpose(1, 0, 2).reshape(128, KC1 * H).astype(BF16_NP)
    )


def _w2_tiles(W):  # [H, H] -> [128, KC2*H]
    return np.ascontiguousarray(
        W.reshape(KC2, 128, H).transpose(1, 0, 2).reshape(128, KC2 * H).astype(BF16_NP)
    )


def kernel(
    field_vectors,
    and_W1,
    and_b1,
    and_W2,
    and_b2,
    or_W1,
    or_b1,
    or_W2,
    or_b2,
    not_W,
    not_b,
    not_idx,
    and_pairs,
    or_pairs,
    leftover_idx,
):
    global LAST_RESULT
    fv = np.asarray(field_vectors, dtype=np.float32)
    and_W1 = np.asarray(and_W1, dtype=np.float32)
    and_W2 = np.asarray(and_W2, dtype=np.float32)
    or_W1 = np.asarray(or_W1, dtype=np.float32)
    or_W2 = np.asarray(or_W2, dtype=np.float32)
    not_W = np.asarray(not_W, dtype=np.float32)
    biases = np.stack(
        [
            np.asarray(and_b1, np.float32),
            np.asarray(or_b1, np.float32),
            np.asarray(and_b2, np.float32),
            np.asarray(or_b2, np.float32),
            np.asarray(not_b, np.float32),
        ]
    ).reshape(5, 1, H)
    not_idx = np.asarray(not_idx).astype(np.int64).ravel()
    and_pairs = np.asarray(and_pairs).astype(np.int64).reshape(NAND, 2)
    or_pairs = np.asarray(or_pairs).astype(np.int64).reshape(NOR, 2)
    leftover_idx = np.asarray(leftover_idx).astype(np.int64).ravel()

    not_set = set(not_idx.tolist())
    pair_fields = np.concatenate([and_pairs.ravel(), or_pairs.ravel()])  # 48 fields
    # The reference applies the not-transform before gathering pairs; with the
    # disjoint index structure used by this problem, pair fields are untouched.
    assert not (set(pair_fields.tolist()) & not_set), (
        "pair fields overlapping not_idx not supported"
    )

    shared = {
        "and_w1t": _w1_tiles(and_W1),
        "or_w1t": _w1_tiles(or_W1),
        "and_w2t": _w2_tiles(and_W2),
        "or_w2t": _w2_tiles(or_W2),
        "not_wt": _w2_tiles(not_W),
        "biases": np.ascontiguousarray(biases.astype(BF16_NP)),
        "b1bc": np.ascontiguousarray(
            np.stack(
                [
                    np.broadcast_to(np.asarray(and_b1, np.float32), (128, H)),
                    np.broadcast_to(np.asarray(or_b1, np.float32), (128, H)),
                ]
            )
        ),
        "b2bc": np.ascontiguousarray(
            np.stack(
                [
                    np.broadcast_to(np.asarray(and_b2, np.float32), (128, H)),
                    np.broadcast_to(np.asarray(or_b2, np.float32), (128, H)),
                ]
            )
        ),
        "ident": np.eye(128, dtype=np.float32).astype(BF16_NP),
    }

    in_maps = []
    for c in range(NCORES):
        fv_c = fv[c * BS : (c + 1) * BS]  # [128, 64, 512]
        G = fv_c[:, pair_fields, :]  # [128, 48, 512]
        G = (
            G.reshape(BS, NPAIR, 2, KC2, 128)
            .transpose(1, 4, 2, 3, 0)
            .reshape(NPAIR, 128, 2 * H)
        )
        N = fv_c[:, not_idx, :]  # [128, 8, 512]
        N = (
            N.reshape(BS, NNOT, KC2, 128)
            .transpose(1, 3, 2, 0)
            .reshape(NNOT, 128, H)
        )
        in_maps.append(
            {
                "pairs_in": np.ascontiguousarray(G.astype(BF16_NP)),
                "nots_in": np.ascontiguousarray(N.astype(BF16_NP)),
                **shared,
            }
        )

    nc = _get_nc()
    res = run_bass_kernel_spmd(nc, in_maps, core_ids=list(range(NCORES)), trace=TRACE)
    LAST_RESULT = res
    results = res.results if hasattr(res, "results") else res

    out = np.empty((B, NAND + NOR + len(leftover_idx), H), dtype=np.float32)
    not_slot = {int(f): j for j, f in enumerate(not_idx)}
    for c in range(NCORES):
        dev = results[c]["out"]  # [32, 128, 512]
        rows = slice(c * BS, (c + 1) * BS)
        out[rows, :NAND] = dev[:NAND].transpose(1, 0, 2)
        out[rows, NAND : NAND + NOR] = dev[NAND:NPAIR].transpose(1, 0, 2)
        for pos, f in enumerate(leftover_idx.tolist()):
            col = NAND + NOR + pos
            if f in not_slot:
                out[rows, col] = dev[NPAIR + not_slot[f]]
            else:
                out[rows, col] = fv[rows, f]
    return out



# revision 31
# speedup vs baseline: 2.8224x; 2.8224x over previous
"""Trainium2 Bass kernel for nn_LogicalOperatorFusion.

Semantics (matches the jax reference):
  fv = field_vectors                                  [B=1024, NF=64, H=512] f32
  fv[:, not_idx] = tanh(fv[:, not_idx] @ not_W + not_b)
  cat_and = concat(fv[:, and_pairs[:,0]], fv[:, and_pairs[:,1]], -1)   [B,16,1024]
  fused_and = relu(cat_and @ and_W1 + and_b1) @ and_W2 + and_b2        [B,16,512]
  cat_or  = ... same with or_pairs (8 pairs) ...                       [B,8,512]
  out = concat([fused_and, fused_or, fv[:, leftover_idx]], axis=1)     [B,40,512]

Strategy:
  - Data parallel: batch sharded 8 ways (128 rows/core), weights replicated.
  - Fully transposed on-chip layout: hidden/output units on partitions,
    batch (128) on the free axis.  L1 produces h^T directly, so L2 needs
    no transposes; biases are per-partition and fused into the ACT pass
    (relu/tanh) or a Pool-engine affine pass (L2 output) -- the PE runs
    nothing but the actual GEMM work.
  - fp8 e4m3 DoubleRow matmuls for L1 and L2 (2 K-planes per instruction,
    0.5 cycles/row): weights are pre-scaled by 256 on the host to stay in
    the e4m3 normal range and descaled via the ACT/Pool `scale` operand.
    The NOT branch (tanh fields) stays bf16 -- its output carries much more
    of the result's Frobenius mass, so fp8 there would blow the error gate.
  - Untouched leftover fields never touch the device; the host copies them
    straight from the f32 input.
"""

import sys

for p in ("/opt/trn_rl_repo",):
    if p not in sys.path:
        sys.path.insert(0, p)

import numpy as np
import ml_dtypes

BF16_NP = ml_dtypes.bfloat16
FP8_NP = ml_dtypes.float8_e4m3

import concourse.bacc as bacc
import concourse.mybir as mybir
from concourse.bass_utils import run_bass_kernel_spmd
from concourse.tile import TileContext

B, NF, H = 1024, 64, 512
NCORES = 8
BS = B // NCORES  # 128 rows per core
NAND, NOR, NNOT = 16, 8, 8
NPAIR = NAND + NOR  # 24
NOUT = NPAIR + NNOT  # 32 device output slots
NGRP = 6  # pair DMA groups of 4
F32 = mybir.dt.float32
BF16 = mybir.dt.bfloat16
FP8 = mybir.dt.float8e4
DR = mybir.MatmulPerfMode.DoubleRow

L2_FP8 = True  # False: L2 + h in bf16 (safer error, slower)
WSCALE = 256.0
WDESCALE = 1.0 / WSCALE

TRACE = False  # test.py sets this for profiled runs
LAST_RESULT = None  # BassKernelResults of the last run (for test.py)

_NC = None  # cached traced Bass program


def _build_nc():
    """Trace the per-core Bass program (identical on all 8 cores)."""
    nc = bacc.Bacc("TRN2", target_bir_lowering=False)

    # All DRAM layouts are partition-major (dim matching SBUF partitions
    # first) so DMA access-pattern dims line up 1:1 with the SBUF tiles.
    pairs_in = nc.dram_tensor(
        "pairs_in", [NGRP, 128, 4, 2, 4, 128], FP8, kind="ExternalInput"
    )  # [grp, p, pair-in-grp, plane, kk, batch]: x^T, K = kk*256 + plane*128 + p
    nots_in = nc.dram_tensor(
        "nots_in", [128, NNOT, 4, 128], BF16, kind="ExternalInput"
    )  # [p, field, kc, batch]: x^T, K = kc*128 + p
    w1a_in = nc.dram_tensor("w1a", [128, 4, 2, 4, 128], FP8, kind="ExternalInput")
    w1o_in = nc.dram_tensor("w1o", [128, 4, 2, 4, 128], FP8, kind="ExternalInput")
    # w1: [p, m, plane, kk, col]: 256*W1[kk*256+plane*128+p, m*128+col]
    if L2_FP8:
        w2a_in = nc.dram_tensor("w2a", [128, 4, 2, 2, 128], FP8, kind="ExternalInput")
        w2o_in = nc.dram_tensor("w2o", [128, 4, 2, 2, 128], FP8, kind="ExternalInput")
    else:
        w2a_in = nc.dram_tensor("w2a", [128, 4, 4, 128], BF16, kind="ExternalInput")
        w2o_in = nc.dram_tensor("w2o", [128, 4, 4, 128], BF16, kind="ExternalInput")
    wn_in = nc.dram_tensor("wn", [128, 4, 4, 128], BF16, kind="ExternalInput")
    # wn/bf16 w2: [p, m, kc, col]: W[kc*128+p, m*128+col]
    brow_in = nc.dram_tensor("brow", [1, 3, 2, 512], FP8, kind="ExternalInput")
    # bias rows for the K=1 DoubleRow bias matmuls: [1, grp, plane, col];
    # grp 0=and_b1*256*8 1=or_b1*256*8 2=not_b*8 (both planes identical; the
    # ones moving operand is 1/16 so the two planes sum back to b).
    b2bc_in = nc.dram_tensor("b2bc", [128, 2, 4, 128], BF16, kind="ExternalInput")
    # [p, type, hc, col]: b2_type[hc*128+p] broadcast along col
    out_d = nc.dram_tensor("out", [128, 4, NOUT, 128], BF16, kind="ExternalOutput")
    # [p, hc, slot, batch]: out^T[slot][hc*128+p, n]

    ACT = mybir.ActivationFunctionType
    HT_DT = FP8 if L2_FP8 else BF16

    with TileContext(nc) as tc:
        with (
            tc.tile_pool(name="consts", bufs=1) as consts,
            tc.tile_pool(name="hid", bufs=3) as hpool,
            tc.tile_pool(name="stage", bufs=1) as spool,
            tc.tile_pool(name="ps1", bufs=3, space="PSUM") as ps1,
            tc.tile_pool(name="ps2", bufs=2, space="PSUM") as ps2,
            tc.tile_pool(name="psn", bufs=3, space="PSUM") as psn,
        ):
            # ---- input DMAs, ordered by first-need time --------------------
            # NOT-phase inputs lead and stay fine-grained so the PE never
            # starves during p-state ramp; the big pair/weight transfers
            # stream in behind them.
            brow = consts.tile([1, 3, 2, 512], FP8, tag="brow")
            nc.sync.dma_start(out=brow, in_=brow_in[0, :, :, :])
            ones = consts.tile([1, 2, 128], FP8, tag="ones")
            nc.vector.memset(ones, 0.0625)
            wn = consts.tile([128, 4, 4, 128], BF16, tag="wn")
            nc.sync.dma_start(out=wn[:, 0:1, :, :], in_=wn_in[:, 0:1, :, :])
            # nt fields grouped so each transfer is big enough to keep the
            # DMA engines busy at the ~650ns/DMA issue rate; fields 6,7 are
            # consumed last so their input rides at the back of the stream.
            ntg = consts.tile([128, NNOT, 4, 128], BF16, tag="ntg")
            pg = [
                consts.tile([128, 4, 2, 4, 128], FP8, tag=f"pg{g}", name=f"pg{g}")
                for g in range(NGRP)
            ]
            w1a = consts.tile([128, 4, 2, 4, 128], FP8, tag="w1a")
            w1o = consts.tile([128, 4, 2, 4, 128], FP8, tag="w1o")
            if L2_FP8:
                w2a = consts.tile([128, 4, 2, 2, 128], FP8, tag="w2a")
                w2o = consts.tile([128, 4, 2, 2, 128], FP8, tag="w2o")
            else:
                w2a = consts.tile([128, 4, 4, 128], BF16, tag="w2a")
                w2o = consts.tile([128, 4, 4, 128], BF16, tag="w2o")
            b2bc = consts.tile([128, 2, 4, 128], BF16, tag="b2bc")

            def dma_nt(f0, f1):
                nc.sync.dma_start(
                    out=ntg[:, f0:f1, :, :], in_=nots_in[:, f0:f1, :, :]
                )

            def dma_pg(g):
                nc.sync.dma_start(out=pg[g], in_=pairs_in[g, :, :, :, :, :])

            dma_nt(0, 1)
            nc.sync.dma_start(out=wn[:, 1:4, :, :], in_=wn_in[:, 1:4, :, :])
            dma_nt(1, 4)
            nc.sync.dma_start(out=w1a, in_=w1a_in[:, :, :, :, :])
            dma_pg(0)
            dma_nt(4, 6)
            dma_pg(1)
            nc.sync.dma_start(out=b2bc, in_=b2bc_in[:, :, :, :])
            if L2_FP8:
                nc.sync.dma_start(out=w2a, in_=w2a_in[:, :, :, :, :])
            else:
                nc.sync.dma_start(out=w2a, in_=w2a_in[:, :, :, :])
            dma_pg(2)
            nc.sync.dma_start(out=w1o, in_=w1o_in[:, :, :, :, :])
            dma_pg(3)
            if L2_FP8:
                nc.sync.dma_start(out=w2o, in_=w2o_in[:, :, :, :, :])
            else:
                nc.sync.dma_start(out=w2o, in_=w2o_in[:, :, :, :])
            dma_pg(4)
            dma_pg(5)
            dma_nt(6, 8)
            w1 = {True: w1a, False: w1o}
            w2 = {True: w2a, False: w2o}

            stage = spool.tile([128, 4, NOUT, 128], BF16, tag="stage")

            # ---- NOT fields first (bf16): tanh(x @ Wn + bn) ----------------
            def emit_not(f):
                pn = psn.tile([128, 4, 128], F32, tag="psn")
                for m in range(4):
                    nc.tensor.matmul(
                        pn[:, m, :],
                        brow[:, 2, :, m * 128 : (m + 1) * 128],
                        ones,
                        start=True,
                        stop=False,
                        perf_mode=DR,
                    )
                    for kc in range(4):
                        nc.tensor.matmul(
                            pn[:, m, :],
                            wn[:, m, kc, :],
                            ntg[:, f, kc, :],
                            start=False,
                            stop=(kc == 3),
                        )
                col = 26 + f if f < 6 else 24 + (f - 6)
                nc.scalar.activation(
                    stage[:, :, col, :], pn[:, :, :], ACT.Tanh, bias=0.0
                )

            # ---- pair pipeline: L1 fp8 DoubleRow -> relu -> L2 -------------
            hstate = {}

            def emit_l1(j):
                is_and = j < NAND
                g, jj = divmod(j, 4)
                p1 = ps1.tile([128, 4, 128], F32, tag="ps1")
                for m in range(4):
                    nc.tensor.matmul(
                        p1[:, m, :],
                        brow[:, 0 if is_and else 1, :, m * 128 : (m + 1) * 128],
                        ones,
                        start=True,
                        stop=False,
                        perf_mode=DR,
                    )
                    for kk in range(4):
                        nc.tensor.matmul(
                            p1[:, m, :],
                            w1[is_and][:, m, :, kk, :],
                            pg[g][:, jj, :, kk, :],
                            start=False,
                            stop=(kk == 3),
                            perf_mode=DR,
                        )
                h = hpool.tile([128, 4, 128], HT_DT, tag="ht")
                nc.scalar.activation(
                    h[:, :, :], p1[:, :, :], ACT.Relu, bias=0.0, scale=WDESCALE
                )
                hstate[j] = h

            def emit_l2(j):
                is_and = j < NAND
                h = hstate.pop(j)
                p2 = ps2.tile([128, 4, 128], F32, tag="ps2")
                if L2_FP8:
                    for m in range(4):
                        for kk in range(2):
                            nc.tensor.matmul(
                                p2[:, m, :],
                                w2[is_and][:, m, :, kk, :],
                                h[:, 2 * kk : 2 * kk + 2, :],
                                start=(kk == 0),
                                stop=(kk == 1),
                                perf_mode=DR,
                            )
                else:
                    for m in range(4):
                        for kc in range(4):
                            nc.tensor.matmul(
                                p2[:, m, :],
                                w2[is_and][:, m, kc, :],
                                h[:, kc, :],
                                start=(kc == 0),
                                stop=(kc == 3),
                            )
                # Pool/GPSIMD cannot read PSUM on real hw -- DVE does all of
                # these (it is otherwise idle)
                nc.vector.scalar_tensor_tensor(
                    out=stage[:, :, j, :],
                    in0=p2[:, :, :],
                    scalar=WDESCALE if L2_FP8 else 1.0,
                    in1=b2bc[:, 0 if is_and else 1, :, :],
                    op0=mybir.AluOpType.mult,
                    op1=mybir.AluOpType.add,
                )

            # slots: 0..23 = pairs (and then or), 24..31 = not fields.
            # Output chunks are emitted right after their gating producer so
            # the SP wait-chain never blocks on far-future work and the
            # transfers spread across the whole pair phase.
            def dma_out(s0, s1):
                nc.sync.dma_start(
                    out=out_d[:, :, s0:s1, :], in_=stage[:, :, s0:s1, :]
                )

            # NOT fields 0..3 lead (their inputs arrive first), 4..5 pad the
            # early pair phase, and 6..7 close the schedule -- their matmuls
            # cover the last relu/L2 waits and their short tanh->DMA chain
            # minimizes the tail.
            for f in range(4):
                emit_not(f)
            emit_l1(0)
            emit_l1(1)
            emit_not(4)
            emit_l1(2)
            emit_l2(0)
            emit_l1(3)
            emit_not(5)
            emit_l2(1)
            for j in range(4, NPAIR):
                emit_l1(j)
                emit_l2(j - 2)
                done = j - 2  # highest finished slot
                if done == 3:
                    dma_out(26, NOUT)
                if done in (3, 7, 11, 15, 19):
                    dma_out(done - 3, done + 1)
            dma_out(20, 22)
            emit_not(6)
            emit_l2(NPAIR - 2)
            emit_l2(NPAIR - 1)
            emit_not(7)
            dma_out(22, 26)

    nc.compile()
    return nc


def _get_nc():
    global _NC
    if _NC is None:
        _NC = _build_nc()
    return _NC


def _w1_pack(W):  # [2H, H] -> [128, 4m, 2plane, 4kk, 128col] fp8, scaled
    t = (W.astype(np.float32) * WSCALE).reshape(4, 2, 128, 4, 128)
    # dims: (kk, plane, p, m, col) -> (p, m, plane, kk, col)
    return np.ascontiguousarray(t.transpose(2, 3, 1, 0, 4)).astype(FP8_NP)


def _w2_pack_fp8(W):  # [H, H] -> [128, 4m, 2plane, 2kk, 128col] fp8, scaled
    t = (W.astype(np.float32) * WSCALE).reshape(2, 2, 128, 4, 128)
    return np.ascontiguousarray(t.transpose(2, 3, 1, 0, 4)).astype(FP8_NP)


def _w_pack_bf16(W):  # [H, H] -> [128, 4m, 4kc, 128col] bf16
    t = np.asarray(W, np.float32).reshape(4, 128, 4, 128)
    # (kc, p, m, col) -> (p, m, kc, col)
    return np.ascontiguousarray(t.transpose(1, 2, 0, 3)).astype(BF16_NP)


def _bcol(b):  # [H] -> [128, 4]: b[hc*128 + p]
    return np.ascontiguousarray(np.asarray(b, np.float32).reshape(4, 128).T)


def _brow(b, fact):  # [H] -> [2, 512] fp8 plane pair, each b*fact*8
    r = (np.asarray(b, np.float32) * (fact * 8.0)).astype(FP8_NP)
    return np.stack([r, r])


def kernel(
    field_vectors,
    and_W1,
    and_b1,
    and_W2,
    and_b2,
    or_W1,
    or_b1,
    or_W2,
    or_b2,
    not_W,
    not_b,
    not_idx,
    and_pairs,
    or_pairs,
    leftover_idx,
):
    global LAST_RESULT
    fv = np.asarray(field_vectors, dtype=np.float32)
    not_idx = np.asarray(not_idx).astype(np.int64).ravel()
    and_pairs = np.asarray(and_pairs).astype(np.int64).reshape(NAND, 2)
    or_pairs = np.asarray(or_pairs).astype(np.int64).reshape(NOR, 2)
    leftover_idx = np.asarray(leftover_idx).astype(np.int64).ravel()

    not_set = set(not_idx.tolist())
    pair_fields = np.concatenate([and_pairs.ravel(), or_pairs.ravel()])  # 48
    # The reference applies the not-transform before gathering pairs; with the
    # disjoint index structure used by this problem, pair fields are untouched.
    assert not (set(pair_fields.tolist()) & not_set), (
        "pair fields overlapping not_idx not supported"
    )

    if L2_FP8:
        w2a, w2o = _w2_pack_fp8(and_W2), _w2_pack_fp8(or_W2)
    else:
        w2a, w2o = _w_pack_bf16(and_W2), _w_pack_bf16(or_W2)
    b2bc = np.empty((128, 2, 4, 128), np.float32)
    b2bc[:, 0] = _bcol(and_b2)[:, :, None]
    b2bc[:, 1] = _bcol(or_b2)[:, :, None]
    shared = {
        "w1a": _w1_pack(and_W1),
        "w1o": _w1_pack(or_W1),
        "w2a": w2a,
        "w2o": w2o,
        "wn": _w_pack_bf16(not_W),
        "brow": np.ascontiguousarray(
            np.stack([_brow(and_b1, WSCALE), _brow(or_b1, WSCALE), _brow(not_b, 1.0)])
        )[None],
        "b2bc": np.ascontiguousarray(b2bc).astype(BF16_NP),
    }

    in_maps = []
    for c in range(NCORES):
        rows = fv[c * BS : (c + 1) * BS]  # [128, 64, 512]
        X = rows[:, pair_fields, :].reshape(BS, NGRP, 4, 4, 2, 128)
        # (n, g, jj, kk, plane, p) -> (g, p, jj, plane, kk, n)
        P = np.ascontiguousarray(X.transpose(1, 5, 2, 4, 3, 0)).astype(FP8_NP)
        Xn = rows[:, not_idx, :].reshape(BS, NNOT, 4, 128)
        # (n, f, kc, p) -> (p, f, kc, n)
        N = np.ascontiguousarray(Xn.transpose(3, 1, 2, 0)).astype(BF16_NP)
        in_maps.append({"pairs_in": P, "nots_in": N, **shared})

    nc = _get_nc()
    res = run_bass_kernel_spmd(nc, in_maps, core_ids=list(range(NCORES)), trace=TRACE)
    LAST_RESULT = res
    results = res.results if hasattr(res, "results") else res

    out = np.empty((B, NAND + NOR + len(leftover_idx), H), dtype=np.float32)
    # device stage column for not-field index j (see emit_not)
    not_slot = {
        int(f): (26 + j if j < 6 else 24 + (j - 6)) for j, f in enumerate(not_idx)
    }
    for c in range(NCORES):
        dev = np.asarray(results[c]["out"])  # [128, 4, 32, 128] bf16
        # (p, hc, slot, n) -> (n, slot, hc, p) -> [128, 32, 512]
        dev = (
            dev.astype(np.float32).transpose(3, 2, 1, 0).reshape(BS, NOUT, H)
        )
        rows = slice(c * BS, (c + 1) * BS)
        out[rows, :NPAIR] = dev[:, :NPAIR]
        for pos, f in enumerate(leftover_idx.tolist()):
            col = NPAIR + pos
            if f in not_slot:
                out[rows, col] = dev[:, not_slot[f]]
            else:
                out[rows, col] = fv[rows, f]
    return out


# revision 37
# speedup vs baseline: 2.9031x; 1.0286x over previous
"""Trainium2 Bass kernel for nn_LogicalOperatorFusion.

Semantics (matches the jax reference):
  fv = field_vectors                                  [B=1024, NF=64, H=512] f32
  fv[:, not_idx] = tanh(fv[:, not_idx] @ not_W + not_b)
  cat_and = concat(fv[:, and_pairs[:,0]], fv[:, and_pairs[:,1]], -1)   [B,16,1024]
  fused_and = relu(cat_and @ and_W1 + and_b1) @ and_W2 + and_b2        [B,16,512]
  cat_or  = ... same with or_pairs (8 pairs) ...                       [B,8,512]
  out = concat([fused_and, fused_or, fv[:, leftover_idx]], axis=1)     [B,40,512]

Strategy:
  - Data parallel: batch sharded 8 ways (128 rows/core), weights replicated.
  - Fully transposed on-chip layout: hidden/output units on partitions,
    batch (128) on the free axis.  L1 produces h^T directly, so L2 needs
    no transposes; biases are per-partition and fused into the ACT pass
    (relu/tanh) or a Pool-engine affine pass (L2 output) -- the PE runs
    nothing but the actual GEMM work.
  - fp8 e4m3 DoubleRow matmuls for L1 and L2 (2 K-planes per instruction,
    0.5 cycles/row): weights are pre-scaled by 256 on the host to stay in
    the e4m3 normal range and descaled via the ACT/Pool `scale` operand.
    The NOT branch (tanh fields) stays bf16 -- its output carries much more
    of the result's Frobenius mass, so fp8 there would blow the error gate.
  - Untouched leftover fields never touch the device; the host copies them
    straight from the f32 input.
"""

import sys

for p in ("/opt/trn_rl_repo",):
    if p not in sys.path:
        sys.path.insert(0, p)

import numpy as np
import ml_dtypes

BF16_NP = ml_dtypes.bfloat16
FP8_NP = ml_dtypes.float8_e4m3

import concourse.bacc as bacc
import concourse.mybir as mybir
from concourse.bass_utils import run_bass_kernel_spmd
from concourse.tile import TileContext

B, NF, H = 1024, 64, 512
NCORES = 8
BS = B // NCORES  # 128 rows per core
NAND, NOR, NNOT = 16, 8, 8
NPAIR = NAND + NOR  # 24
NOUT = NPAIR + NNOT  # 32 device output slots
NGRP = 6  # pair DMA groups of 4
F32 = mybir.dt.float32
BF16 = mybir.dt.bfloat16
FP8 = mybir.dt.float8e4
DR = mybir.MatmulPerfMode.DoubleRow

L2_FP8 = True  # False: L2 + h in bf16 (safer error, slower)
WSCALE = 256.0
WDESCALE = 1.0 / WSCALE

TRACE = False  # test.py sets this for profiled runs
LAST_RESULT = None  # BassKernelResults of the last run (for test.py)

_NC = None  # cached traced Bass program


def _build_nc():
    """Trace the per-core Bass program (identical on all 8 cores)."""
    nc = bacc.Bacc("TRN2", target_bir_lowering=False)

    # All DRAM layouts are partition-major (dim matching SBUF partitions
    # first) so DMA access-pattern dims line up 1:1 with the SBUF tiles.
    pairs_in = nc.dram_tensor(
        "pairs_in", [NGRP, 128, 4, 2, 4, 128], FP8, kind="ExternalInput"
    )  # [grp, p, pair-in-grp, plane, kk, batch]: x^T, K = kk*256 + plane*128 + p
    nots_in = nc.dram_tensor(
        "nots_in", [128, NNOT, 4, 128], BF16, kind="ExternalInput"
    )  # [p, field, kc, batch]: x^T, K = kc*128 + p
    w1a_in = nc.dram_tensor("w1a", [128, 4, 2, 4, 128], FP8, kind="ExternalInput")
    w1o_in = nc.dram_tensor("w1o", [128, 4, 2, 4, 128], FP8, kind="ExternalInput")
    # w1: [p, m, plane, kk, col]: 256*W1[kk*256+plane*128+p, m*128+col]
    if L2_FP8:
        w2a_in = nc.dram_tensor("w2a", [128, 4, 2, 2, 128], FP8, kind="ExternalInput")
        w2o_in = nc.dram_tensor("w2o", [128, 4, 2, 2, 128], FP8, kind="ExternalInput")
    else:
        w2a_in = nc.dram_tensor("w2a", [128, 4, 4, 128], BF16, kind="ExternalInput")
        w2o_in = nc.dram_tensor("w2o", [128, 4, 4, 128], BF16, kind="ExternalInput")
    wn_in = nc.dram_tensor("wn", [128, 4, 4, 128], BF16, kind="ExternalInput")
    # wn/bf16 w2: [p, m, kc, col]: W[kc*128+p, m*128+col]
    brow_in = nc.dram_tensor("brow", [1, 3, 2, 512], FP8, kind="ExternalInput")
    # bias rows for the K=1 DoubleRow bias matmuls: [1, grp, plane, col];
    # grp 0=and_b1*256*8 1=or_b1*256*8 2=not_b*8 (both planes identical; the
    # ones moving operand is 1/16 so the two planes sum back to b).
    b2bc_in = nc.dram_tensor("b2bc", [128, 2, 4, 128], BF16, kind="ExternalInput")
    # [p, type, hc, col]: b2_type[hc*128+p] broadcast along col
    out_d = nc.dram_tensor("out", [128, 4, NOUT, 128], BF16, kind="ExternalOutput")
    # [p, hc, slot, batch]: out^T[slot][hc*128+p, n]

    ACT = mybir.ActivationFunctionType
    HT_DT = FP8 if L2_FP8 else BF16

    with TileContext(nc) as tc:
        with (
            tc.tile_pool(name="consts", bufs=1) as consts,
            tc.tile_pool(name="hid", bufs=3) as hpool,
            tc.tile_pool(name="stage", bufs=1) as spool,
            tc.tile_pool(name="ps1", bufs=3, space="PSUM") as ps1,
            tc.tile_pool(name="ps2", bufs=2, space="PSUM") as ps2,
            tc.tile_pool(name="psn", bufs=3, space="PSUM") as psn,
        ):
            # ---- input DMAs, ordered by first-need time --------------------
            # NOT-phase inputs lead and stay fine-grained so the PE never
            # starves during p-state ramp; the big pair/weight transfers
            # stream in behind them.
            brow = consts.tile([1, 3, 2, 512], FP8, tag="brow")
            nc.sync.dma_start(out=brow, in_=brow_in[0, :, :, :])
            ones = consts.tile([1, 2, 128], FP8, tag="ones")
            nc.vector.memset(ones, 0.0625)
            wn = consts.tile([128, 4, 4, 128], BF16, tag="wn")
            nc.sync.dma_start(out=wn, in_=wn_in[:, :, :, :])
            # nt fields grouped so each transfer is big enough to keep the
            # DMA engines busy at the ~650ns/DMA issue rate; fields 6,7 are
            # consumed last so their input rides at the back of the stream.
            ntg = consts.tile([128, NNOT, 4, 128], BF16, tag="ntg")
            pg = [
                consts.tile([128, 4, 2, 4, 128], FP8, tag=f"pg{g}", name=f"pg{g}")
                for g in range(NGRP)
            ]
            w1a = consts.tile([128, 4, 2, 4, 128], FP8, tag="w1a")
            w1o = consts.tile([128, 4, 2, 4, 128], FP8, tag="w1o")
            if L2_FP8:
                w2a = consts.tile([128, 4, 2, 2, 128], FP8, tag="w2a")
                w2o = consts.tile([128, 4, 2, 2, 128], FP8, tag="w2o")
            else:
                w2a = consts.tile([128, 4, 4, 128], BF16, tag="w2a")
                w2o = consts.tile([128, 4, 4, 128], BF16, tag="w2o")
            b2bc = consts.tile([128, 2, 4, 128], BF16, tag="b2bc")

            def dma_nt(f0, f1):
                nc.sync.dma_start(
                    out=ntg[:, f0:f1, :, :], in_=nots_in[:, f0:f1, :, :]
                )

            def dma_pg(g):
                nc.sync.dma_start(out=pg[g], in_=pairs_in[g, :, :, :, :, :])

            dma_nt(0, 1)
            dma_nt(1, 2)
            dma_nt(2, 4)
            nc.sync.dma_start(out=w1a, in_=w1a_in[:, :, :, :, :])
            dma_pg(0)
            dma_nt(4, 6)
            dma_pg(1)
            nc.sync.dma_start(out=b2bc, in_=b2bc_in[:, :, :, :])
            if L2_FP8:
                nc.sync.dma_start(out=w2a, in_=w2a_in[:, :, :, :, :])
            else:
                nc.sync.dma_start(out=w2a, in_=w2a_in[:, :, :, :])
            dma_pg(2)
            nc.sync.dma_start(out=w1o, in_=w1o_in[:, :, :, :, :])
            dma_pg(3)
            if L2_FP8:
                nc.sync.dma_start(out=w2o, in_=w2o_in[:, :, :, :, :])
            else:
                nc.sync.dma_start(out=w2o, in_=w2o_in[:, :, :, :])
            dma_pg(4)
            dma_pg(5)
            dma_nt(6, 8)
            w1 = {True: w1a, False: w1o}
            w2 = {True: w2a, False: w2o}

            stage = spool.tile([128, 4, NOUT, 128], BF16, tag="stage")

            # ---- NOT fields first (bf16): tanh(x @ Wn + bn) ----------------
            def emit_not(f):
                pn = psn.tile([128, 4, 128], F32, tag="psn")
                for m in range(4):
                    nc.tensor.matmul(
                        pn[:, m, :],
                        brow[:, 2, :, m * 128 : (m + 1) * 128],
                        ones,
                        start=True,
                        stop=False,
                        perf_mode=DR,
                    )
                    for kc in range(4):
                        nc.tensor.matmul(
                            pn[:, m, :],
                            wn[:, m, kc, :],
                            ntg[:, f, kc, :],
                            start=False,
                            stop=(kc == 3),
                        )
                col = 26 + f if f < 6 else 24 + (f - 6)
                nc.scalar.activation(
                    stage[:, :, col, :], pn[:, :, :], ACT.Tanh, bias=0.0
                )

            # ---- pair pipeline: L1 fp8 DoubleRow -> relu -> L2 -------------
            hstate = {}

            def emit_l1(j):
                is_and = j < NAND
                g, jj = divmod(j, 4)
                p1 = ps1.tile([128, 4, 128], F32, tag="ps1")
                for m in range(4):
                    nc.tensor.matmul(
                        p1[:, m, :],
                        brow[:, 0 if is_and else 1, :, m * 128 : (m + 1) * 128],
                        ones,
                        start=True,
                        stop=False,
                        perf_mode=DR,
                    )
                    for kk in range(4):
                        nc.tensor.matmul(
                            p1[:, m, :],
                            w1[is_and][:, m, :, kk, :],
                            pg[g][:, jj, :, kk, :],
                            start=False,
                            stop=(kk == 3),
                            perf_mode=DR,
                        )
                h = hpool.tile([128, 4, 128], HT_DT, tag="ht")
                nc.scalar.activation(
                    h[:, :, :], p1[:, :, :], ACT.Relu, bias=0.0, scale=WDESCALE
                )
                hstate[j] = h

            def emit_l2(j):
                is_and = j < NAND
                h = hstate.pop(j)
                p2 = ps2.tile([128, 4, 128], F32, tag="ps2")
                if L2_FP8:
                    for m in range(4):
                        for kk in range(2):
                            nc.tensor.matmul(
                                p2[:, m, :],
                                w2[is_and][:, m, :, kk, :],
                                h[:, 2 * kk : 2 * kk + 2, :],
                                start=(kk == 0),
                                stop=(kk == 1),
                                perf_mode=DR,
                            )
                else:
                    for m in range(4):
                        for kc in range(4):
                            nc.tensor.matmul(
                                p2[:, m, :],
                                w2[is_and][:, m, kc, :],
                                h[:, kc, :],
                                start=(kc == 0),
                                stop=(kc == 3),
                            )
                # Pool/GPSIMD cannot read PSUM on real hw -- DVE does all of
                # these (it is otherwise idle)
                nc.vector.scalar_tensor_tensor(
                    out=stage[:, :, j, :],
                    in0=p2[:, :, :],
                    scalar=WDESCALE if L2_FP8 else 1.0,
                    in1=b2bc[:, 0 if is_and else 1, :, :],
                    op0=mybir.AluOpType.mult,
                    op1=mybir.AluOpType.add,
                )

            # slots: 0..23 = pairs (and then or), 24..31 = not fields.
            # Output chunks are emitted right after their gating producer so
            # the SP wait-chain never blocks on far-future work and the
            # transfers spread across the whole pair phase.
            def dma_out(s0, s1):
                nc.sync.dma_start(
                    out=out_d[:, :, s0:s1, :], in_=stage[:, :, s0:s1, :]
                )

            # NOT fields 0..3 lead (their inputs arrive first), 4..5 pad the
            # early pair phase, and 6..7 close the schedule -- their matmuls
            # cover the last relu/L2 waits and their short tanh->DMA chain
            # minimizes the tail.
            for f in range(4):
                emit_not(f)
            emit_l1(0)
            emit_l1(1)
            emit_not(4)
            emit_l1(2)
            emit_l2(0)
            emit_l1(3)
            emit_not(5)
            emit_l2(1)
            for j in range(4, NPAIR):
                emit_l1(j)
                emit_l2(j - 2)
                done = j - 2  # highest finished slot
                if done == 3:
                    dma_out(26, NOUT)
                if done in (3, 7, 11, 15, 19):
                    dma_out(done - 3, done + 1)
            dma_out(20, 22)
            emit_not(6)
            emit_l2(NPAIR - 2)
            emit_l2(NPAIR - 1)
            emit_not(7)
            dma_out(22, 24)
            # tanh(6)/tanh(7) land last; issuing their chunk from the (now
            # idle) ACT engine lets its sem wait run parallel to SP's
            nc.scalar.dma_start(out=out_d[:, :, 24:26, :], in_=stage[:, :, 24:26, :])

    nc.compile()
    return nc


def _get_nc():
    global _NC
    if _NC is None:
        _NC = _build_nc()
    return _NC


def _w1_pack(W):  # [2H, H] -> [128, 4m, 2plane, 4kk, 128col] fp8, scaled
    t = (W.astype(np.float32) * WSCALE).reshape(4, 2, 128, 4, 128)
    # dims: (kk, plane, p, m, col) -> (p, m, plane, kk, col)
    return np.ascontiguousarray(t.transpose(2, 3, 1, 0, 4)).astype(FP8_NP)


def _w2_pack_fp8(W):  # [H, H] -> [128, 4m, 2plane, 2kk, 128col] fp8, scaled
    t = (W.astype(np.float32) * WSCALE).reshape(2, 2, 128, 4, 128)
    return np.ascontiguousarray(t.transpose(2, 3, 1, 0, 4)).astype(FP8_NP)


def _w_pack_bf16(W):  # [H, H] -> [128, 4m, 4kc, 128col] bf16
    t = np.asarray(W, np.float32).reshape(4, 128, 4, 128)
    # (kc, p, m, col) -> (p, m, kc, col)
    return np.ascontiguousarray(t.transpose(1, 2, 0, 3)).astype(BF16_NP)


def _bcol(b):  # [H] -> [128, 4]: b[hc*128 + p]
    return np.ascontiguousarray(np.asarray(b, np.float32).reshape(4, 128).T)


def _brow(b, fact):  # [H] -> [2, 512] fp8 plane pair, each b*fact*8
    r = (np.asarray(b, np.float32) * (fact * 8.0)).astype(FP8_NP)
    return np.stack([r, r])


def kernel(
    field_vectors,
    and_W1,
    and_b1,
    and_W2,
    and_b2,
    or_W1,
    or_b1,
    or_W2,
    or_b2,
    not_W,
    not_b,
    not_idx,
    and_pairs,
    or_pairs,
    leftover_idx,
):
    global LAST_RESULT
    fv = np.asarray(field_vectors, dtype=np.float32)
    not_idx = np.asarray(not_idx).astype(np.int64).ravel()
    and_pairs = np.asarray(and_pairs).astype(np.int64).reshape(NAND, 2)
    or_pairs = np.asarray(or_pairs).astype(np.int64).reshape(NOR, 2)
    leftover_idx = np.asarray(leftover_idx).astype(np.int64).ravel()

    not_set = set(not_idx.tolist())
    pair_fields = np.concatenate([and_pairs.ravel(), or_pairs.ravel()])  # 48
    # The reference applies the not-transform before gathering pairs; with the
    # disjoint index structure used by this problem, pair fields are untouched.
    assert not (set(pair_fields.tolist()) & not_set), (
        "pair fields overlapping not_idx not supported"
    )

    if L2_FP8:
        w2a, w2o = _w2_pack_fp8(and_W2), _w2_pack_fp8(or_W2)
    else:
        w2a, w2o = _w_pack_bf16(and_W2), _w_pack_bf16(or_W2)
    b2bc = np.empty((128, 2, 4, 128), np.float32)
    b2bc[:, 0] = _bcol(and_b2)[:, :, None]
    b2bc[:, 1] = _bcol(or_b2)[:, :, None]
    shared = {
        "w1a": _w1_pack(and_W1),
        "w1o": _w1_pack(or_W1),
        "w2a": w2a,
        "w2o": w2o,
        "wn": _w_pack_bf16(not_W),
        "brow": np.ascontiguousarray(
            np.stack([_brow(and_b1, WSCALE), _brow(or_b1, WSCALE), _brow(not_b, 1.0)])
        )[None],
        "b2bc": np.ascontiguousarray(b2bc).astype(BF16_NP),
    }

    in_maps = []
    for c in range(NCORES):
        rows = fv[c * BS : (c + 1) * BS]  # [128, 64, 512]
        X = rows[:, pair_fields, :].reshape(BS, NGRP, 4, 4, 2, 128)
        # (n, g, jj, kk, plane, p) -> (g, p, jj, plane, kk, n)
        P = np.ascontiguousarray(X.transpose(1, 5, 2, 4, 3, 0)).astype(FP8_NP)
        Xn = rows[:, not_idx, :].reshape(BS, NNOT, 4, 128)
        # (n, f, kc, p) -> (p, f, kc, n)
        N = np.ascontiguousarray(Xn.transpose(3, 1, 2, 0)).astype(BF16_NP)
        in_maps.append({"pairs_in": P, "nots_in": N, **shared})

    nc = _get_nc()
    res = run_bass_kernel_spmd(nc, in_maps, core_ids=list(range(NCORES)), trace=TRACE)
    LAST_RESULT = res
    results = res.results if hasattr(res, "results") else res

    out = np.empty((B, NAND + NOR + len(leftover_idx), H), dtype=np.float32)
    # device stage column for not-field index j (see emit_not)
    not_slot = {
        int(f): (26 + j if j < 6 else 24 + (j - 6)) for j, f in enumerate(not_idx)
    }
    for c in range(NCORES):
        dev = np.asarray(results[c]["out"])  # [128, 4, 32, 128] bf16
        # (p, hc, slot, n) -> (n, slot, hc, p) -> [128, 32, 512]
        dev = (
            dev.astype(np.float32).transpose(3, 2, 1, 0).reshape(BS, NOUT, H)
        )
        rows = slice(c * BS, (c + 1) * BS)
        out[rows, :NPAIR] = dev[:, :NPAIR]
        for pos, f in enumerate(leftover_idx.tolist()):
            col = NPAIR + pos
            if f in not_slot:
                out[rows, col] = dev[:, not_slot[f]]
            else:
                out[rows, col] = fv[rows, f]
    return out
